# revision 8
# baseline (speedup 1.0000x reference)
"""CrossTeacherAttention Trainium2 kernel, v2 (engine-balanced exp design).

Math per batch element b (x as [C=256, N=1024], N=H*W):
  G  = M Xs + gb,  M = Wk^T Wq, gb = Wk^T bq   (host, fp8-packed input)
  S_t[m,n] = sum_c Xt[c,m] G[c,n]              (PE, fp8 DoubleRow, f32 PSUM)
  E_t = ~exp(S_t/16 - 0.5) as e5m2, two flavors per-tile:
    ACT: native table exp (scale=1/16, bias=-0.5) -> e5m2
    DVE: one-op Schraudolph straight to e5m2 BITS:
         bits = rint(A8*S + B8) as uint8, bitcast e5m2
         (A8 = 4/(16 ln2); B8 = 60 - 2/ln2 - 4c; convert is
          round-to-nearest + saturate, so negative tails clamp to 0)
  V_t^T aug = [Xt^T Wv^T | 3.0]                 (host, fp8 input; col 256
         makes O[:,256] = 3*Z_t = denominator * inverse teacher weight)
  O-pair p (nk=2p,2p+1): [128, 2, 512] PSUM, cols 0:257 used; 8 fp8 DR
         matmuls accumulate E^T V over the 4 m-pair chunks.
  combine: ACT/DVE pair-copy O -> SBUF f32 tmp [128,2,257]; DVE recip of
         tmp[:,:,256]; Pool (SBUF-only engine) does tmp*rp -> bf16 and
         acc += that; acc arrives preloaded with Xs^T + bv.
  out = acc (bf16), DMA'd per-pair as teacher-2 combines land.

Engine balance targets: ACT ~13 exps + ~6 pair-copies, DVE ~11 exps +
~6 pair-copies + recips, Pool all 48 combine ops + some input DMA
issuance, PE 144 matmuls (~10.6us), SP most DMA issuance.

Sharding: data-parallel over batch, B=8 -> one batch element per core.
"""

import sys

sys.path.insert(0, "/opt/trn_rl_repo")

import ml_dtypes
import numpy as np

import concourse.bass as bass
import concourse.tile as tile
from concourse import mybir
from concourse.bass_utils import run_bass_kernel_spmd

B, C, H, W = 8, 256, 32, 32
N = H * W  # 1024
T = 3
P = 128
F32 = mybir.dt.float32
BF16 = mybir.dt.bfloat16
F8 = mybir.dt.float8e4
F8E5 = mybir.dt.float8e5
U8 = mybir.dt.uint8
NP_F8 = ml_dtypes.float8_e4m3
NP_BF16 = ml_dtypes.bfloat16
SCALE = C ** -0.5  # 1/16
EXP_BIAS = -0.5
C_SCH = 0.0579
A8 = 4.0 / (16.0 * np.log(2.0))
B8 = 60.0 + 4.0 * EXP_BIAS / np.log(2.0) - 4.0 * C_SCH
DR = mybir.MatmulPerfMode.DoubleRow

# exp engine assignment per (t, mi): listed mi run on ACT (native exp),
# the rest on DVE (one-op Schraudolph). Consistent odd/even parity keeps
# both engines fed from the 3-slot S rotation without transition stalls;
# one extra ACT exp (t1 mi0) because ACT is faster per op.
ACT_EXP = {
    0: [1, 3, 5, 7],
    1: [0, 1, 3, 5, 7],
    2: [1, 3, 5, 7],
}
# pair-copy engine per (t, p): listed p run on ACT
ACT_COPY = {
    0: [0, 1, 3],
    1: [0, 1, 3],
    2: [3],
}


def build_nc():
    nc = bass.Bass()
    gf_d = nc.dram_tensor("gf", [P, 2, N], F8, kind="ExternalInput")
    xt0_d = nc.dram_tensor("xt0", [P, 2, N], F8, kind="ExternalInput")
    xt12_d = nc.dram_tensor("xt12", [P, 2, 2, N], F8, kind="ExternalInput")
    vt0_d = nc.dram_tensor("vt0", [P, 4, 2, 257], F8, kind="ExternalInput")
    vt12_d = nc.dram_tensor("vt12", [P, 2, 4, 2, 257], F8,
                            kind="ExternalInput")
    acc_d = nc.dram_tensor("accin", [P, 8, C], BF16, kind="ExternalInput")
    out_d = nc.dram_tensor("out", [P, 8, C], BF16, kind="ExternalOutput")

    with tile.TileContext(nc) as tc:
        with (
            tc.tile_pool(name="consts", bufs=1) as consts,
            tc.tile_pool(name="epool", bufs=12) as epool,
            tc.tile_pool(name="rpool", bufs=16) as rpool,
            tc.tile_pool(name="ps", bufs=3, space="PSUM") as ps,
            tc.tile_pool(name="po", bufs=1, space="PSUM") as po,
        ):
            # ---- warm-up first: ACT queue must stay clear so the Exp
            # table load finishes by ~1.5us ----
            warm = consts.tile([P, 1], F32, tag="warm", name="warm")
            nc.vector.memset(warm, 0.0)
            ebias = consts.tile([P, 1], F32, tag="ebias", name="ebias")
            nc.vector.memset(ebias, EXP_BIAS)
            nc.scalar.activation(
                warm, warm, func=mybir.ActivationFunctionType.Exp)
            # ---- input DMAs: SP carries gf/acc/vt12, Pool carries
            # xt0/vt0/xt12; ACT carries none ----
            gf = consts.tile([P, 2, N], F8, tag="gf", name="gf")
            nc.sync.dma_start(out=gf, in_=gf_d[:, :, :])
            xt0 = consts.tile([P, 2, N], F8, tag="xt0", name="xt0")
            nc.gpsimd.dma_start(out=xt0, in_=xt0_d[:, :, :])
            vt0 = consts.tile([P, 4, 2, 257], F8, tag="vt0", name="vt0")
            nc.gpsimd.dma_start(out=vt0, in_=vt0_d[:, :, :, :])
            xt12 = consts.tile([P, 2, 2, N], F8, tag="xt12", name="xt12")
            nc.gpsimd.dma_start(out=xt12, in_=xt12_d[:, :, :, :])
            acc = consts.tile([P, 8, C], BF16, tag="acc", name="acc")
            nc.sync.dma_start(out=acc, in_=acc_d[:, :, :])
            vt12 = consts.tile([P, 2, 4, 2, 257], F8, tag="vt12",
                               name="vt12")
            nc.sync.dma_start(out=vt12, in_=vt12_d[:, :, :, :, :])

            def xt(t):
                return xt0 if t == 0 else xt12[:, t - 1]

            def vt(t, r):
                return vt0[:, r] if t == 0 else vt12[:, t - 1, r]

            e_tiles = [[None] * 4 for _ in range(T)]

            def emit_smm(t, mi):
                sp = ps.tile([P, N], F32, tag="s", name=f"sp{t}{mi}")
                for nh in range(2):
                    nc.tensor.matmul(
                        sp[:, nh * 512:(nh + 1) * 512],
                        xt(t)[:, :, mi * P:(mi + 1) * P],
                        gf[:, :, nh * 512:(nh + 1) * 512],
                        start=True, stop=True, perf_mode=DR,
                    )
                return sp

            def emit_exp(t, mi, sp, cols=slice(0, N)):
                r, j = divmod(mi, 2)
                if e_tiles[t][r] is None:
                    e_tiles[t][r] = epool.tile([P, 2, N], F8E5, tag="e",
                                               name=f"e{t}{r}")
                if mi in ACT_EXP[t]:
                    nc.scalar.activation(
                        e_tiles[t][r][:, j, cols], sp[:, cols],
                        func=mybir.ActivationFunctionType.Exp,
                        bias=ebias[:, 0:1], scale=SCALE,
                    )
                else:
                    nc.vector.tensor_scalar(
                        out=e_tiles[t][r][:, j, cols].bitcast(U8),
                        in0=sp[:, cols],
                        scalar1=A8, scalar2=B8,
                        op0=mybir.AluOpType.mult, op1=mybir.AluOpType.add,
                    )

            def emit_opair(t, p, rs=range(4), pool=None, tag="o"):
                """O matmuls for nk pair (2p, 2p+1); returns the pair tile."""
                op = (pool or po).tile([P, 2, 512], F32, tag=tag,
                                       name=f"o{t}{p}")
                for r in rs:
                    for j in range(2):
                        nk = 2 * p + j
                        nc.tensor.matmul(
                            op[:, j, :257],
                            e_tiles[t][r][:, :, nk * P:(nk + 1) * P],
                            vt(t, r),
                            start=(r == 0), stop=(r == 3), perf_mode=DR,
                        )
                return op

            def emit_combine(t, p, op, direct=False):
                if direct:
                    # all-DVE: recip of the PSUM Z column + two stt's that
                    # read O straight from PSUM (no copy, no Pool)
                    rp = rpool.tile([P, 2], F32, tag="rp", name=f"rp{t}{p}")
                    nc.vector.reciprocal(rp, op[:, :, 256])
                    for j in range(2):
                        nk = 2 * p + j
                        nc.vector.scalar_tensor_tensor(
                            out=acc[:, nk, :], in0=op[:, j, 0:256],
                            scalar=rp[:, j:j + 1], in1=acc[:, nk, :],
                            op0=mybir.AluOpType.mult,
                            op1=mybir.AluOpType.add,
                        )
                else:
                    tmp = rpool.tile([P, 2, 257], F32, tag="tmp",
                                     name=f"tmp{t}{p}")
                    if p in ACT_COPY[t]:
                        nc.scalar.activation(
                            tmp, op[:, :, 0:257],
                            func=mybir.ActivationFunctionType.Copy)
                    else:
                        nc.vector.tensor_copy(tmp, op[:, :, 0:257])
                    rp = rpool.tile([P, 2], F32, tag="rp", name=f"rp{t}{p}")
                    nc.vector.reciprocal(rp, tmp[:, :, 256])
                    for j in range(2):
                        nk = 2 * p + j
                        tmp2 = rpool.tile([P, C], BF16, tag="tmp2",
                                          name=f"tmp2{t}{nk}")
                        nc.gpsimd.tensor_scalar(
                            out=tmp2, in0=tmp[:, j, 0:256],
                            scalar1=rp[:, j:j + 1], scalar2=None,
                            op0=mybir.AluOpType.mult,
                        )
                        nc.gpsimd.tensor_tensor(
                            out=acc[:, nk, :], in0=tmp2, in1=acc[:, nk, :],
                            op=mybir.AluOpType.add,
                        )
                if t == 2:
                    # issue from Pool: no cross-engine hop after its tt
                    eng = nc.sync if direct else nc.gpsimd
                    eng.dma_start(out=out_d[:, 2 * p:2 * p + 2, :],
                                  in_=acc[:, 2 * p:2 * p + 2, :])

            # ---- schedule ----
            # teacher 0: S+exp straight through; the first tile's exp is
            # split by n-halves so it can start as soon as the first
            # gf-half DMA lands
            sps = {}
            for mi in range(8):
                sps[(0, mi)] = emit_smm(0, mi)
                if mi == 0:
                    emit_exp(0, 0, sps[(0, 0)], slice(0, 512))
                    emit_exp(0, 0, sps[(0, 0)], slice(512, N))
                else:
                    emit_exp(0, mi, sps[(0, mi)])
            # teacher 1 S+exp, interleaving teacher-0 O pairs + combines
            for mi in range(8):
                sps[(1, mi)] = emit_smm(1, mi)
                emit_exp(1, mi, sps[(1, mi)])
                if mi % 2 == 1:
                    p = mi // 2
                    op = emit_opair(0, p)
                    emit_combine(0, p, op)
            # teacher 2 S+exp, interleaving teacher-1 O pairs + combines;
            # the last exp pair is split by n-halves so teacher-2 O r3
            # matmuls for early pairs can run under the tail exps
            for mi in range(8):
                sps[(2, mi)] = emit_smm(2, mi)
                if mi < 6:
                    emit_exp(2, mi, sps[(2, mi)])
                if mi % 2 == 1:
                    p = mi // 2
                    op = emit_opair(1, p)
                    emit_combine(1, p, op)
            # tail: halves of mi6 (DVE) and mi7 (ACT) let pairs 0,1 drain
            # while the second halves run; pairs 2,3 finish on parallel
            # engine paths (pair2 DVE-direct stt, pair3 ACT copy + Pool)
            emit_exp(2, 6, sps[(2, 6)], slice(0, 512))
            emit_exp(2, 7, sps[(2, 7)], slice(0, 512))
            op0 = emit_opair(2, 0)
            op1 = emit_opair(2, 1, pool=ps, tag="s")
            emit_exp(2, 6, sps[(2, 6)], slice(512, N))
            emit_combine(2, 0, op0)
            emit_exp(2, 7, sps[(2, 7)], slice(512, N))
            emit_combine(2, 1, op1)
            op2 = emit_opair(2, 2, pool=ps, tag="s")
            op3 = emit_opair(2, 3, pool=ps, tag="s")
            emit_combine(2, 2, op2, direct=True)
            emit_combine(2, 3, op3)

    _split_multi_waits(nc)
    if not nc.is_finalized():
        nc.finalize()
    return nc


def _split_multi_waits(nc):
    """walrus can encode at most one sync-wait per instruction. Hoist every
    wait of a multi-wait instruction onto single-wait nops on the same
    engine, placed immediately before it in program order."""
    fixes = []
    for fn in nc.m.functions:
        for blk in fn.blocks:
            for inst in blk.instructions:
                si = getattr(inst, "sync_info", None)
                if (si is not None and si.on_wait and len(si.on_wait) > 1
                        and getattr(inst, "engine", None) is not None):
                    fixes.append((blk, inst))
    for blk, inst in fixes:
        si = inst.sync_info
        waits = list(si.on_wait)
        nops = []
        for w in waits:
            nop = nc.engines[inst.engine].nop(nofuse=True).ins
            nop.sync_info = mybir.SyncInfo(on_wait=[w], on_update=[])
            nops.append(nop)
        inst.sync_info = mybir.SyncInfo(on_wait=[], on_update=list(si.on_update))
        nop_names = {n.name for n in nops}
        for fn2 in nc.m.functions:
            for blk2 in fn2.blocks:
                blk2.instructions = [
                    i for i in blk2.instructions if i.name not in nop_names
                ]
        pos = next(i for i, x in enumerate(blk.instructions)
                   if x.name == inst.name)
        blk.instructions = (blk.instructions[:pos] + nops
                            + blk.instructions[pos:])


_NC = None


def _get_nc():
    global _NC
    if _NC is None:
        _NC = build_nc()
    return _NC


def _pack2(a):
    """[256, X] row-major -> [128, 2, X] with row c at [c % 128, c // 128]."""
    return np.ascontiguousarray(a.reshape(2, P, -1).transpose(1, 0, 2))


def _pack_v(v_aug):
    """[N=1024, 257] -> [128, 4, 2, 257]: vt[p, r, j, c] = V[r*256+j*128+p]."""
    return np.ascontiguousarray(
        v_aug.reshape(4, 2, P, 257).transpose(2, 0, 1, 3))


def make_in_maps(student_feat, t_feat0, t_feat1, t_feat2,
                 Wq, bq, Wk, bk, Wv, bv):
    xs = np.asarray(student_feat, np.float32).reshape(B, C, N)
    xt = np.ascontiguousarray(
        np.stack([t_feat0, t_feat1, t_feat2], axis=1), np.float32
    ).reshape(B, T, C, N)
    wq32 = np.asarray(Wq, np.float32)
    wk32 = np.asarray(Wk, np.float32)
    wv32 = np.asarray(Wv, np.float32)
    m = wk32.T @ wq32
    gb = wk32.T @ np.asarray(bq, np.float32)
    bv32 = np.asarray(bv, np.float32)

    maps = []
    ones = np.full((N, 1), 3.0, np.float32)
    for b in range(B):
        gf = _pack2((m @ xs[b] + gb[:, None]).astype(NP_F8))
        xq = xt[b].astype(NP_F8)  # [T, C, N]
        xt0 = _pack2(xq[0])
        xt12 = np.stack([_pack2(xq[1]), _pack2(xq[2])], axis=1)
        vts = []
        for t in range(T):
            v_aug = np.concatenate(
                [xt[b, t].T @ wv32.T, ones], axis=1).astype(NP_F8)
            vts.append(_pack_v(v_aug))
        vt0 = vts[0]
        vt12 = np.stack([vts[1], vts[2]], axis=1)
        accin = np.ascontiguousarray(
            (xs[b].T + bv32[None, :]).reshape(8, P, C).transpose(1, 0, 2)
        ).astype(NP_BF16)
        maps.append({"gf": gf, "xt0": xt0, "xt12": xt12, "vt0": vt0,
                     "vt12": vt12, "accin": accin})
    return maps


def run(in_maps, trace=False):
    nc = _get_nc()
    return run_bass_kernel_spmd(nc, in_maps, core_ids=list(range(B)),
                                trace=trace)


def unpack_out(raw):
    """[128, 8, 256] bf16 n-major -> [C, H, W] f32."""
    o = np.asarray(raw).astype(np.float32).transpose(1, 0, 2).reshape(N, C)
    return np.ascontiguousarray(o.T).reshape(C, H, W)


def kernel(student_feat, t_feat0, t_feat1, t_feat2,
           Wq, bq, Wk, bk, Wv, bv):
    in_maps = make_in_maps(student_feat, t_feat0, t_feat1, t_feat2,
                           Wq, bq, Wk, bk, Wv, bv)
    res = None
    for attempt in range(3):
        try:
            res = run(in_maps, trace=False)
            break
        except Exception:
            if attempt == 2:
                raise
    out = np.stack([unpack_out(res.results[b]["out"]) for b in range(B)])
    return out.astype(np.float32)


# revision 9
# speedup vs baseline: 1.0029x; 1.0029x over previous
"""CrossTeacherAttention Trainium2 kernel, v2 (engine-balanced exp design).

Math per batch element b (x as [C=256, N=1024], N=H*W):
  G  = M Xs + gb,  M = Wk^T Wq, gb = Wk^T bq   (host, fp8-packed input)
  S_t[m,n] = sum_c Xt[c,m] G[c,n]              (PE, fp8 DoubleRow, f32 PSUM)
  E_t = ~exp(S_t/16 - 0.5) as e5m2, two flavors per-tile:
    ACT: native table exp (scale=1/16, bias=-0.5) -> e5m2
    DVE: one-op Schraudolph straight to e5m2 BITS:
         bits = rint(A8*S + B8) as uint8, bitcast e5m2
         (A8 = 4/(16 ln2); B8 = 60 - 2/ln2 - 4c; convert is
          round-to-nearest + saturate, so negative tails clamp to 0)
  V_t^T aug = [Xt^T Wv^T | 3.0]                 (host, fp8 input; col 256
         makes O[:,256] = 3*Z_t = denominator * inverse teacher weight)
  O-pair p (nk=2p,2p+1): [128, 2, 512] PSUM, cols 0:257 used; 8 fp8 DR
         matmuls accumulate E^T V over the 4 m-pair chunks.
  combine: ACT/DVE pair-copy O -> SBUF f32 tmp [128,2,257]; DVE recip of
         tmp[:,:,256]; Pool (SBUF-only engine) does tmp*rp -> bf16 and
         acc += that; acc arrives preloaded with Xs^T + bv.
  out = acc (bf16), DMA'd per-pair as teacher-2 combines land.

Engine balance targets: ACT ~13 exps + ~6 pair-copies, DVE ~11 exps +
~6 pair-copies + recips, Pool all 48 combine ops + some input DMA
issuance, PE 144 matmuls (~10.6us), SP most DMA issuance.

Sharding: data-parallel over batch, B=8 -> one batch element per core.
"""

import sys

sys.path.insert(0, "/opt/trn_rl_repo")

import ml_dtypes
import numpy as np

import concourse.bass as bass
import concourse.tile as tile
from concourse import mybir
from concourse.bass_utils import run_bass_kernel_spmd

B, C, H, W = 8, 256, 32, 32
N = H * W  # 1024
T = 3
P = 128
F32 = mybir.dt.float32
BF16 = mybir.dt.bfloat16
F8 = mybir.dt.float8e4
F8E5 = mybir.dt.float8e5
U8 = mybir.dt.uint8
NP_F8 = ml_dtypes.float8_e4m3
NP_BF16 = ml_dtypes.bfloat16
SCALE = C ** -0.5  # 1/16
EXP_BIAS = -0.5
C_SCH = 0.0579
A8 = 4.0 / (16.0 * np.log(2.0))
B8 = 60.0 + 4.0 * EXP_BIAS / np.log(2.0) - 4.0 * C_SCH
DR = mybir.MatmulPerfMode.DoubleRow

# exp engine assignment per (t, mi): listed mi run on ACT (native exp),
# the rest on DVE (one-op Schraudolph). Consistent odd/even parity keeps
# both engines fed from the 3-slot S rotation without transition stalls;
# one extra ACT exp (t1 mi0) because ACT is faster per op.
ACT_EXP = {
    0: [1, 3, 5, 7],
    1: [0, 1, 3, 5, 7],
    2: [0, 1, 3, 5, 7],
}
# pair-copy engine per (t, p): listed p run on ACT
ACT_COPY = {
    0: [1, 3],
    1: [1, 3],
    2: [3],
}


def build_nc():
    nc = bass.Bass()
    gf_d = nc.dram_tensor("gf", [P, 2, N], F8, kind="ExternalInput")
    xt0_d = nc.dram_tensor("xt0", [P, 2, N], F8, kind="ExternalInput")
    xt12_d = nc.dram_tensor("xt12", [P, 2, 2, N], F8, kind="ExternalInput")
    vt0_d = nc.dram_tensor("vt0", [P, 4, 2, 257], F8, kind="ExternalInput")
    vt12_d = nc.dram_tensor("vt12", [P, 2, 4, 2, 257], F8,
                            kind="ExternalInput")
    acc_d = nc.dram_tensor("accin", [P, 8, C], BF16, kind="ExternalInput")
    out_d = nc.dram_tensor("out", [P, 8, C], BF16, kind="ExternalOutput")

    with tile.TileContext(nc) as tc:
        with (
            tc.tile_pool(name="consts", bufs=1) as consts,
            tc.tile_pool(name="epool", bufs=12) as epool,
            tc.tile_pool(name="rpool", bufs=16) as rpool,
            tc.tile_pool(name="ps", bufs=3, space="PSUM") as ps,
            tc.tile_pool(name="po", bufs=1, space="PSUM") as po,
        ):
            # ---- warm-up first: ACT queue must stay clear so the Exp
            # table load finishes by ~1.5us ----
            warm = consts.tile([P, 1], F32, tag="warm", name="warm")
            nc.vector.memset(warm, 0.0)
            ebias = consts.tile([P, 1], F32, tag="ebias", name="ebias")
            nc.vector.memset(ebias, EXP_BIAS)
            nc.scalar.activation(
                warm, warm, func=mybir.ActivationFunctionType.Exp)
            # ---- input DMAs: SP carries gf/acc/vt12, Pool carries
            # xt0/vt0/xt12; ACT carries none ----
            gf = consts.tile([P, 2, N], F8, tag="gf", name="gf")
            nc.sync.dma_start(out=gf, in_=gf_d[:, :, :])
            xt0 = consts.tile([P, 2, N], F8, tag="xt0", name="xt0")
            nc.gpsimd.dma_start(out=xt0, in_=xt0_d[:, :, :])
            vt0 = consts.tile([P, 4, 2, 257], F8, tag="vt0", name="vt0")
            nc.gpsimd.dma_start(out=vt0, in_=vt0_d[:, :, :, :])
            xt12 = consts.tile([P, 2, 2, N], F8, tag="xt12", name="xt12")
            nc.gpsimd.dma_start(out=xt12, in_=xt12_d[:, :, :, :])
            acc = consts.tile([P, 8, C], BF16, tag="acc", name="acc")
            nc.sync.dma_start(out=acc, in_=acc_d[:, :, :])
            vt12 = consts.tile([P, 2, 4, 2, 257], F8, tag="vt12",
                               name="vt12")
            nc.sync.dma_start(out=vt12, in_=vt12_d[:, :, :, :, :])

            def xt(t):
                return xt0 if t == 0 else xt12[:, t - 1]

            def vt(t, r):
                return vt0[:, r] if t == 0 else vt12[:, t - 1, r]

            e_tiles = [[None] * 4 for _ in range(T)]

            def emit_smm(t, mi):
                sp = ps.tile([P, N], F32, tag="s", name=f"sp{t}{mi}")
                for nh in range(2):
                    nc.tensor.matmul(
                        sp[:, nh * 512:(nh + 1) * 512],
                        xt(t)[:, :, mi * P:(mi + 1) * P],
                        gf[:, :, nh * 512:(nh + 1) * 512],
                        start=True, stop=True, perf_mode=DR,
                    )
                return sp

            def emit_exp(t, mi, sp, cols=slice(0, N), out=None):
                r, j = divmod(mi, 2)
                if out is None:
                    if e_tiles[t][r] is None:
                        e_tiles[t][r] = epool.tile([P, 2, N], F8E5,
                                                   tag="e", name=f"e{t}{r}")
                    out = e_tiles[t][r][:, j, cols]
                if mi in ACT_EXP[t]:
                    nc.scalar.activation(
                        out, sp[:, cols],
                        func=mybir.ActivationFunctionType.Exp,
                        bias=ebias[:, 0:1], scale=SCALE,
                    )
                else:
                    nc.vector.tensor_scalar(
                        out=out.bitcast(U8),
                        in0=sp[:, cols],
                        scalar1=A8, scalar2=B8,
                        op0=mybir.AluOpType.mult, op1=mybir.AluOpType.add,
                    )

            def emit_opair(t, p, rs=range(4), pool=None, tag="o",
                           e3=None, e3_base=0):
                """O matmuls for nk pair (2p, 2p+1); returns the pair tile.
                e3: optional half-tile override for the r=3 stationary
                (its columns start at e3_base)."""
                op = (pool or po).tile([P, 2, 512], F32, tag=tag,
                                       name=f"o{t}{p}")
                for r in rs:
                    for j in range(2):
                        nk = 2 * p + j
                        if r == 3 and e3 is not None:
                            stat = e3[:, :, nk * P - e3_base:
                                      (nk + 1) * P - e3_base]
                        else:
                            stat = e_tiles[t][r][:, :, nk * P:(nk + 1) * P]
                        nc.tensor.matmul(
                            op[:, j, :257],
                            stat,
                            vt(t, r),
                            start=(r == 0), stop=(r == 3), perf_mode=DR,
                        )
                return op

            def emit_combine(t, p, op, direct=False):
                if direct:
                    # all-DVE: recip of the PSUM Z column + two stt's that
                    # read O straight from PSUM (no copy, no Pool)
                    rp = rpool.tile([P, 2], F32, tag="rp", name=f"rp{t}{p}")
                    nc.vector.reciprocal(rp, op[:, :, 256])
                    for j in range(2):
                        nk = 2 * p + j
                        nc.vector.scalar_tensor_tensor(
                            out=acc[:, nk, :], in0=op[:, j, 0:256],
                            scalar=rp[:, j:j + 1], in1=acc[:, nk, :],
                            op0=mybir.AluOpType.mult,
                            op1=mybir.AluOpType.add,
                        )
                else:
                    tmp = rpool.tile([P, 2, 257], F32, tag="tmp",
                                     name=f"tmp{t}{p}")
                    if p in ACT_COPY[t]:
                        nc.scalar.activation(
                            tmp, op[:, :, 0:257],
                            func=mybir.ActivationFunctionType.Copy)
                    else:
                        nc.vector.tensor_copy(tmp, op[:, :, 0:257])
                    rp = rpool.tile([P, 2], F32, tag="rp", name=f"rp{t}{p}")
                    nc.vector.reciprocal(rp, tmp[:, :, 256])
                    for j in range(2):
                        nk = 2 * p + j
                        tmp2 = rpool.tile([P, C], BF16, tag="tmp2",
                                          name=f"tmp2{t}{nk}")
                        nc.gpsimd.tensor_scalar(
                            out=tmp2, in0=tmp[:, j, 0:256],
                            scalar1=rp[:, j:j + 1], scalar2=None,
                            op0=mybir.AluOpType.mult,
                        )
                        nc.gpsimd.tensor_tensor(
                            out=acc[:, nk, :], in0=tmp2, in1=acc[:, nk, :],
                            op=mybir.AluOpType.add,
                        )
                if t == 2:
                    # issue from Pool: no cross-engine hop after its tt
                    eng = nc.sync if direct else nc.gpsimd
                    eng.dma_start(out=out_d[:, 2 * p:2 * p + 2, :],
                                  in_=acc[:, 2 * p:2 * p + 2, :])

            # ---- schedule ----
            # teacher 0: S+exp straight through; the first tile's exp is
            # split by n-halves so it can start as soon as the first
            # gf-half DMA lands
            sps = {}
            for mi in range(8):
                sps[(0, mi)] = emit_smm(0, mi)
                if mi == 0:
                    emit_exp(0, 0, sps[(0, 0)], slice(0, 512))
                    emit_exp(0, 0, sps[(0, 0)], slice(512, N))
                else:
                    emit_exp(0, mi, sps[(0, mi)])
            # teacher 1 S+exp, interleaving teacher-0 O pairs + combines
            for mi in range(8):
                sps[(1, mi)] = emit_smm(1, mi)
                emit_exp(1, mi, sps[(1, mi)])
                if mi % 2 == 1:
                    p = mi // 2
                    op = emit_opair(0, p)
                    emit_combine(0, p, op)
            # teacher 2 S+exp, interleaving teacher-1 O pairs + combines;
            # the last exp pair is split by n-halves so teacher-2 O r3
            # matmuls for early pairs can run under the tail exps
            for mi in range(8):
                sps[(2, mi)] = emit_smm(2, mi)
                if mi < 6:
                    emit_exp(2, mi, sps[(2, mi)])
                if mi % 2 == 1:
                    p = mi // 2
                    op = emit_opair(1, p)
                    emit_combine(1, p, op)
            # tail: mi6/mi7 exps split by n-halves into SEPARATE half
            # tiles so pairs 0,1 (cols 0:512) drain while the second
            # halves run; pairs 2,3 finish on parallel engine paths
            # (pair2 DVE-direct stt, pair3 ACT copy + Pool)
            e3a = epool.tile([P, 2, 512], F8E5, tag="e3a", name="e3a")
            e3b = epool.tile([P, 2, 512], F8E5, tag="e3b", name="e3b")
            emit_exp(2, 6, sps[(2, 6)], slice(0, 512),
                     out=e3a[:, 0, :])
            emit_exp(2, 7, sps[(2, 7)], slice(0, 512),
                     out=e3a[:, 1, :])
            op0 = emit_opair(2, 0, e3=e3a, e3_base=0)
            op1 = emit_opair(2, 1, pool=ps, tag="s", e3=e3a, e3_base=0)
            emit_exp(2, 6, sps[(2, 6)], slice(512, N),
                     out=e3b[:, 0, :])
            emit_combine(2, 0, op0)
            emit_exp(2, 7, sps[(2, 7)], slice(512, N),
                     out=e3b[:, 1, :])
            emit_combine(2, 1, op1)
            op2 = emit_opair(2, 2, pool=ps, tag="s", e3=e3b, e3_base=512)
            op3 = emit_opair(2, 3, pool=ps, tag="s", e3=e3b, e3_base=512)
            emit_combine(2, 2, op2, direct=True)
            emit_combine(2, 3, op3)

    _split_multi_waits(nc)
    if not nc.is_finalized():
        nc.finalize()
    return nc


def _split_multi_waits(nc):
    """walrus can encode at most one sync-wait per instruction. Hoist every
    wait of a multi-wait instruction onto single-wait nops on the same
    engine, placed immediately before it in program order."""
    fixes = []
    for fn in nc.m.functions:
        for blk in fn.blocks:
            for inst in blk.instructions:
                si = getattr(inst, "sync_info", None)
                if (si is not None and si.on_wait and len(si.on_wait) > 1
                        and getattr(inst, "engine", None) is not None):
                    fixes.append((blk, inst))
    for blk, inst in fixes:
        si = inst.sync_info
        waits = list(si.on_wait)
        nops = []
        for w in waits:
            nop = nc.engines[inst.engine].nop(nofuse=True).ins
            nop.sync_info = mybir.SyncInfo(on_wait=[w], on_update=[])
            nops.append(nop)
        inst.sync_info = mybir.SyncInfo(on_wait=[], on_update=list(si.on_update))
        nop_names = {n.name for n in nops}
        for fn2 in nc.m.functions:
            for blk2 in fn2.blocks:
                blk2.instructions = [
                    i for i in blk2.instructions if i.name not in nop_names
                ]
        pos = next(i for i, x in enumerate(blk.instructions)
                   if x.name == inst.name)
        blk.instructions = (blk.instructions[:pos] + nops
                            + blk.instructions[pos:])


_NC = None


def _get_nc():
    global _NC
    if _NC is None:
        _NC = build_nc()
    return _NC


def _pack2(a):
    """[256, X] row-major -> [128, 2, X] with row c at [c % 128, c // 128]."""
    return np.ascontiguousarray(a.reshape(2, P, -1).transpose(1, 0, 2))


def _pack_v(v_aug):
    """[N=1024, 257] -> [128, 4, 2, 257]: vt[p, r, j, c] = V[r*256+j*128+p]."""
    return np.ascontiguousarray(
        v_aug.reshape(4, 2, P, 257).transpose(2, 0, 1, 3))


def make_in_maps(student_feat, t_feat0, t_feat1, t_feat2,
                 Wq, bq, Wk, bk, Wv, bv):
    xs = np.asarray(student_feat, np.float32).reshape(B, C, N)
    xt = np.ascontiguousarray(
        np.stack([t_feat0, t_feat1, t_feat2], axis=1), np.float32
    ).reshape(B, T, C, N)
    wq32 = np.asarray(Wq, np.float32)
    wk32 = np.asarray(Wk, np.float32)
    wv32 = np.asarray(Wv, np.float32)
    m = wk32.T @ wq32
    gb = wk32.T @ np.asarray(bq, np.float32)
    bv32 = np.asarray(bv, np.float32)

    maps = []
    ones = np.full((N, 1), 3.0, np.float32)
    for b in range(B):
        gf = _pack2((m @ xs[b] + gb[:, None]).astype(NP_F8))
        xq = xt[b].astype(NP_F8)  # [T, C, N]
        xt0 = _pack2(xq[0])
        xt12 = np.stack([_pack2(xq[1]), _pack2(xq[2])], axis=1)
        vts = []
        for t in range(T):
            v_aug = np.concatenate(
                [xt[b, t].T @ wv32.T, ones], axis=1).astype(NP_F8)
            vts.append(_pack_v(v_aug))
        vt0 = vts[0]
        vt12 = np.stack([vts[1], vts[2]], axis=1)
        accin = np.ascontiguousarray(
            (xs[b].T + bv32[None, :]).reshape(8, P, C).transpose(1, 0, 2)
        ).astype(NP_BF16)
        maps.append({"gf": gf, "xt0": xt0, "xt12": xt12, "vt0": vt0,
                     "vt12": vt12, "accin": accin})
    return maps


def run(in_maps, trace=False):
    nc = _get_nc()
    return run_bass_kernel_spmd(nc, in_maps, core_ids=list(range(B)),
                                trace=trace)


def unpack_out(raw):
    """[128, 8, 256] bf16 n-major -> [C, H, W] f32."""
    o = np.asarray(raw).astype(np.float32).transpose(1, 0, 2).reshape(N, C)
    return np.ascontiguousarray(o.T).reshape(C, H, W)


def kernel(student_feat, t_feat0, t_feat1, t_feat2,
           Wq, bq, Wk, bk, Wv, bv):
    in_maps = make_in_maps(student_feat, t_feat0, t_feat1, t_feat2,
                           Wq, bq, Wk, bk, Wv, bv)
    res = None
    for attempt in range(3):
        try:
            res = run(in_maps, trace=False)
            break
        except Exception:
            if attempt == 2:
                raise
    out = np.stack([unpack_out(res.results[b]["out"]) for b in range(B)])
    return out.astype(np.float32)
